# revision 18
# baseline (speedup 1.0000x reference)
"""CavityLoss Trainium2 kernel (nn_CavityLoss_43722767073667), v10.

Mathematical reduction of the reference, exact in fp32 (verified against a
bit-faithful numpy emulation incl. adversarial threshold-boundary values):

  pb = (floor(pred*255) >= 128)  <=>  (pred >= c*),  c* = f32(128/255)
  The 5^3 all-ones dilation of the binary gt is an exact integer count
  >= gt (the window contains the center voxel), so
      diff = ((gt - pb*dilate(gt)) > 0) == gt * (1 - pb)     [identity]
  Non-critical voxels contribute exactly 0 to the BCE in fp32, so
      loss = -mean( gt * [pred < c*] * ln(pred) ).

Per-tile chain (v10 "accumulate-on-DVE"):
  STT#1 (DVE): r    = (p is_ge c*) max p    # p if p < c*, else exactly 1.0
  Ln    (ACT): ln_r = Ln(r)                 # gated by pred only
  STT#2 (DVE): acc[:,k] = sum((gt bypass 0) mult ln_r)   # accum_out row-sum
  gt==0 voxels contribute exact 0; gt==1, p>=c* contribute Ln_table(1.0)
  (<=4e-7); residual <= ~1.3e-6 relative, far under the 2e-2 gate.

Design notes (from HW traces of v6 @35.6us through v10 @33.0us):
  - The measured window is [first const-memset .. last walrus postamble
    instruction]; the ~6.8us all-semaphore-reset postamble (48 resets at
    122ns on the PE sequencer are the tail) and ~1us preamble are
    walrus-fixed, so only the body (lead-in + stream + tail) is optimizable.
    (A PE "p-state warmup" attempt made every engine's reset cadence WORSE;
    reverted.)
  - The input stream is HBM/DMA-engine bound; SDMA engine 15 is persistently
    ~13% slower (21.4 vs 24.7 B/ns) and its FIFO backlog sets every
    transfer's completion-sem time (~1.5ns/col cumulative). Partial-partition
    DMAs (to starve engine 15's partitions) backfire on BOTH DGE paths: the
    descriptor->engine assignment chunks onto 4 engines and the whole stream
    slows ~25% (v8). So the layout stays uniform [128, 6912].
  - ACT work (Ln) is pred-gated: with pred tiles leading their gt partner by
    LEAD transfer slots, every Ln lands mid-stream; the post-stream tail is
    just the last accumulating STTs. No ACT READ_ACCUMULATOR chain (v6),
    no PE matmul / PSUM copy: ACT itself DMAs acc[128,NT] on its own HWDGE
    ring right after the last accum lands; host reduces in f64. No wait on
    the out-DMA sem: the walrus postamble outlasts the landing by ~6us.
  - Tile sizes: descending ramp from a discrete-event model of the tail
    (eng15 clock, DVE/ACT serialization, measured op costs) so each
    accum-STT finishes roughly when the next gt sem fires. Measured
    tradeoff: more/smaller transfers shorten the tail but add ~0.8us of
    stream overhead (16 vs 10 transfers), netting ~zero between the 5-tile
    and 8-tile configs; s_fin is co-bound by the eng15 stream clock and
    DVE stt2 throughput.

Distribution: 192^3 volume flattened and split into 8 equal slabs (depth
sharding: 24 z-planes per core), each viewed as [128 partitions, 6912].
Pointwise + reduction only - the dilation cancels, so no halo exchange and
no collectives.
"""

import numpy as np

import concourse.bacc as bacc
import concourse.mybir as mybir
from concourse.bass_utils import run_bass_kernel_spmd

D = 192
N_CORES = 8
P = 128
TOTAL = D * D * D              # 7_077_888
PER_CORE = TOTAL // N_CORES    # 884_736
FREE = PER_CORE // P           # 6_912
SIZES = [1792, 1408, 1024, 768, 576, 512, 448, 384]
LEAD = 3                       # pred tiles lead their gt partner by this many
assert sum(SIZES) == FREE
NT = len(SIZES)

C_STAR = float(np.float32(128.0) / np.float32(255.0))

_CACHE = {}


def _build():
    nc = bacc.Bacc("TRN2", name="cavity_loss")
    f32 = mybir.dt.float32
    pred = nc.dram_tensor("pred", [P, FREE], f32, kind="ExternalInput")
    gt = nc.dram_tensor("gt", [P, FREE], f32, kind="ExternalInput")
    out = nc.dram_tensor("out", [P, NT], f32, kind="ExternalOutput")

    ge = mybir.AluOpType.is_ge
    byp = mybir.AluOpType.bypass
    mult = mybir.AluOpType.mult
    mx = mybir.AluOpType.max
    Ln = mybir.ActivationFunctionType.Ln

    pred_sb = nc.alloc_sbuf_tensor("pred_sb", [P, FREE], f32).ap()
    gt_sb = nc.alloc_sbuf_tensor("gt_sb", [P, FREE], f32).ap()
    r_sb = nc.alloc_sbuf_tensor("r_sb", [P, FREE], f32).ap()
    ln_sb = nc.alloc_sbuf_tensor("ln_sb", [P, FREE], f32).ap()
    acc = nc.alloc_sbuf_tensor("acc_sb", [P, NT], f32).ap()

    s_pred = [nc.alloc_semaphore(f"s_pred{t}") for t in range(NT)]
    s_gt = [nc.alloc_semaphore(f"s_gt{t}") for t in range(NT)]
    s_r1 = nc.alloc_semaphore("s_r1")
    s_ln = nc.alloc_semaphore("s_ln")
    s_fin = nc.alloc_semaphore("s_fin")
    s_out = nc.alloc_semaphore("s_out")

    offs = np.concatenate([[0], np.cumsum(SIZES)]).tolist()
    sls = [slice(offs[t], offs[t + 1]) for t in range(NT)]

    # single sync HWDGE ring; pred tiles lead their gt partner by LEAD slots
    # so STT#1+Ln always complete before the gt-gated accumulating STT#2
    def dma_p(t):
        nc.sync.dma_start(pred_sb[:, sls[t]], pred[:, sls[t]]).then_inc(
            s_pred[t], 16
        )

    def dma_g(t):
        nc.sync.dma_start(gt_sb[:, sls[t]], gt[:, sls[t]]).then_inc(s_gt[t], 16)

    for t in range(LEAD):
        dma_p(t)
    for t in range(LEAD, NT):
        dma_g(t - LEAD)
        dma_p(t)
    for t in range(NT - LEAD, NT):
        dma_g(t)

    # scalar (ACT): dummy Ln hoists the ACT_TABLE_LOAD into the DMA window,
    # then the pred-gated Ln chain; finalize DMA on ACT's own HWDGE ring
    dummy = nc.alloc_sbuf_tensor("dummy_sb", [P, 1], f32).ap()
    nc.scalar.activation(dummy[:], nc.const_aps.tensor(1.0, (P, 1)), Ln)
    for t in range(NT):
        nc.scalar.wait_ge(s_r1, t + 1)
        nc.scalar.activation(ln_sb[:, sls[t]], r_sb[:, sls[t]], Ln).then_inc(
            s_ln, 1
        )
    nc.scalar.wait_ge(s_fin, 1)
    # no wait on s_out: nothing consumes it, and the walrus postamble (the
    # ~6us all-semaphore-reset chain + final drain/barrier) outlasts the
    # 2.5KB out-DMA landing by several us, so the write is safely retired
    # before NEFF completion; dropping the wait starts the postamble ~1.5us
    # earlier, inside the measured window
    nc.scalar.dma_start(out[:], acc[:]).then_inc(s_out, 16)

    # vector (DVE): STT#1 (pred-keyed) and accumulating STT#2 (gt-keyed)
    def stt1(t):
        sl = sls[t]
        nc.vector.wait_ge(s_pred[t], 16)
        nc.vector.scalar_tensor_tensor(
            r_sb[:, sl], pred_sb[:, sl], C_STAR, pred_sb[:, sl], ge, mx
        ).then_inc(s_r1, 1)

    def stt2(t):
        sl = sls[t]
        nc.vector.wait_ge(s_gt[t], 16)
        nc.vector.wait_ge(s_ln, t + 1)
        return nc.vector.scalar_tensor_tensor(
            r_sb[:, sl], gt_sb[:, sl], 0.0, ln_sb[:, sl], byp, mult,
            accum_out=acc[:, t : t + 1],
        )

    last = None
    for t in range(LEAD):
        stt1(t)
    for t in range(LEAD, NT):
        last = stt2(t - LEAD)
        stt1(t)
    for t in range(NT - LEAD, NT):
        last = stt2(t)
    last.then_inc(s_fin, 1)

    nc.compile()
    return nc


def _get_nc():
    if "nc" not in _CACHE:
        _CACHE["nc"] = _build()
    return _CACHE["nc"]


def _shard(x):
    flat = np.ascontiguousarray(np.asarray(x, dtype=np.float32)).reshape(-1)
    assert flat.size == TOTAL, f"expected {TOTAL} elements, got {flat.size}"
    return [
        flat[c * PER_CORE : (c + 1) * PER_CORE].reshape(P, FREE)
        for c in range(N_CORES)
    ]


def run_spmd(pred, gt, **kw):
    """Shard, run on 8 cores; returns BassKernelResults (kw e.g. trace=True)."""
    preds = _shard(pred)
    gts = _shard(gt)
    in_maps = [{"pred": preds[c], "gt": gts[c]} for c in range(N_CORES)]
    return run_bass_kernel_spmd(
        _get_nc(), in_maps, core_ids=list(range(N_CORES)), **kw
    )


def kernel(pred, gt):
    res = run_spmd(pred, gt)
    total = 0.0
    for r in res.results:
        total += float(r["out"].astype(np.float64).sum())
    return np.asarray(np.float32(-total / TOTAL))


# revision 19
# speedup vs baseline: 1.0482x; 1.0482x over previous
"""CavityLoss Trainium2 kernel (nn_CavityLoss_43722767073667), v10.

Mathematical reduction of the reference, exact in fp32 (verified against a
bit-faithful numpy emulation incl. adversarial threshold-boundary values):

  pb = (floor(pred*255) >= 128)  <=>  (pred >= c*),  c* = f32(128/255)
  The 5^3 all-ones dilation of the binary gt is an exact integer count
  >= gt (the window contains the center voxel), so
      diff = ((gt - pb*dilate(gt)) > 0) == gt * (1 - pb)     [identity]
  Non-critical voxels contribute exactly 0 to the BCE in fp32, so
      loss = -mean( gt * [pred < c*] * ln(pred) ).

Per-tile chain (v10 "accumulate-on-DVE"):
  STT#1 (DVE): r    = (p is_ge c*) max p    # p if p < c*, else exactly 1.0
  Ln    (ACT): ln_r = Ln(r)                 # gated by pred only
  STT#2 (DVE): acc[:,k] = sum((gt bypass 0) mult ln_r)   # accum_out row-sum
  gt==0 voxels contribute exact 0; gt==1, p>=c* contribute Ln_table(1.0)
  (<=4e-7); residual <= ~1.3e-6 relative, far under the 2e-2 gate.

Design notes (from HW traces of v6 @35.6us through v10 @33.0us):
  - The measured window is [first const-memset .. last walrus postamble
    instruction]; the ~6.8us all-semaphore-reset postamble (48 resets at
    122ns on the PE sequencer are the tail) and ~1us preamble are
    walrus-fixed, so only the body (lead-in + stream + tail) is optimizable.
    (A PE "p-state warmup" attempt made every engine's reset cadence WORSE;
    reverted.)
  - The input stream is HBM/DMA-engine bound; SDMA engine 15 is persistently
    ~13% slower (21.4 vs 24.7 B/ns) and its FIFO backlog sets every
    transfer's completion-sem time (~1.5ns/col cumulative). Partial-partition
    DMAs (to starve engine 15's partitions) backfire on BOTH DGE paths: the
    descriptor->engine assignment chunks onto 4 engines and the whole stream
    slows ~25% (v8). So the layout stays uniform [128, 6912].
  - ACT work (Ln) is pred-gated: with pred tiles leading their gt partner by
    LEAD transfer slots, every Ln lands mid-stream; the post-stream tail is
    just the last accumulating STTs. No ACT READ_ACCUMULATOR chain (v6),
    no PE matmul / PSUM copy: ACT itself DMAs acc[128,NT] on its own HWDGE
    ring right after the last accum lands; host reduces in f64. No wait on
    the out-DMA sem: the walrus postamble outlasts the landing by ~6us.
  - Tile sizes: descending ramp from a discrete-event model of the tail
    (eng15 clock, DVE/ACT serialization, measured op costs) so each
    accum-STT finishes roughly when the next gt sem fires. Measured
    tradeoff: more/smaller transfers shorten the tail but add ~0.8us of
    stream overhead (16 vs 10 transfers), netting ~zero between the 5-tile
    and 8-tile configs; s_fin is co-bound by the eng15 stream clock and
    DVE stt2 throughput.

Distribution: 192^3 volume flattened and split into 8 equal slabs (depth
sharding: 24 z-planes per core), each viewed as [128 partitions, 6912].
Pointwise + reduction only - the dilation cancels, so no halo exchange and
no collectives.
"""

import numpy as np

import concourse.bacc as bacc
import concourse.mybir as mybir
from concourse.bass_utils import run_bass_kernel_spmd

D = 192
N_CORES = 8
P = 128
TOTAL = D * D * D              # 7_077_888
PER_CORE = TOTAL // N_CORES    # 884_736
FREE = PER_CORE // P           # 6_912
SIZES = [2048, 1536, 1152, 896, 704, 576]
LEAD = 2                       # pred tiles lead their gt partner by this many
assert sum(SIZES) == FREE
NT = len(SIZES)

C_STAR = float(np.float32(128.0) / np.float32(255.0))

_CACHE = {}


def _build():
    nc = bacc.Bacc("TRN2", name="cavity_loss")
    f32 = mybir.dt.float32
    pred = nc.dram_tensor("pred", [P, FREE], f32, kind="ExternalInput")
    gt = nc.dram_tensor("gt", [P, FREE], f32, kind="ExternalInput")
    out = nc.dram_tensor("out", [P, NT], f32, kind="ExternalOutput")

    ge = mybir.AluOpType.is_ge
    byp = mybir.AluOpType.bypass
    mult = mybir.AluOpType.mult
    mx = mybir.AluOpType.max
    Ln = mybir.ActivationFunctionType.Ln

    pred_sb = nc.alloc_sbuf_tensor("pred_sb", [P, FREE], f32).ap()
    gt_sb = nc.alloc_sbuf_tensor("gt_sb", [P, FREE], f32).ap()
    r_sb = nc.alloc_sbuf_tensor("r_sb", [P, FREE], f32).ap()
    ln_sb = nc.alloc_sbuf_tensor("ln_sb", [P, FREE], f32).ap()
    acc = nc.alloc_sbuf_tensor("acc_sb", [P, NT], f32).ap()

    s_pred = [nc.alloc_semaphore(f"s_pred{t}") for t in range(NT)]
    s_gt = [nc.alloc_semaphore(f"s_gt{t}") for t in range(NT)]
    s_r1 = nc.alloc_semaphore("s_r1")
    s_ln = nc.alloc_semaphore("s_ln")
    s_fin = nc.alloc_semaphore("s_fin")
    s_out = nc.alloc_semaphore("s_out")

    offs = np.concatenate([[0], np.cumsum(SIZES)]).tolist()
    sls = [slice(offs[t], offs[t + 1]) for t in range(NT)]

    # single sync HWDGE ring; pred tiles lead their gt partner by LEAD slots
    # so STT#1+Ln always complete before the gt-gated accumulating STT#2
    def dma_p(t):
        nc.sync.dma_start(pred_sb[:, sls[t]], pred[:, sls[t]]).then_inc(
            s_pred[t], 16
        )

    def dma_g(t):
        nc.sync.dma_start(gt_sb[:, sls[t]], gt[:, sls[t]]).then_inc(s_gt[t], 16)

    for t in range(LEAD):
        dma_p(t)
    for t in range(LEAD, NT):
        dma_g(t - LEAD)
        dma_p(t)
    for t in range(NT - LEAD, NT):
        dma_g(t)

    # scalar (ACT): dummy Ln hoists the ACT_TABLE_LOAD into the DMA window,
    # then the pred-gated Ln chain; finalize DMA on ACT's own HWDGE ring
    dummy = nc.alloc_sbuf_tensor("dummy_sb", [P, 1], f32).ap()
    nc.scalar.activation(dummy[:], nc.const_aps.tensor(1.0, (P, 1)), Ln)
    for t in range(NT):
        nc.scalar.wait_ge(s_r1, t + 1)
        nc.scalar.activation(ln_sb[:, sls[t]], r_sb[:, sls[t]], Ln).then_inc(
            s_ln, 1
        )
    nc.scalar.wait_ge(s_fin, 1)
    # no wait on s_out: nothing consumes it, and the walrus postamble (the
    # ~6us all-semaphore-reset chain + final drain/barrier) outlasts the
    # 2.5KB out-DMA landing by several us, so the write is safely retired
    # before NEFF completion; dropping the wait starts the postamble ~1.5us
    # earlier, inside the measured window
    nc.scalar.dma_start(out[:], acc[:]).then_inc(s_out, 16)

    # vector (DVE): STT#1 (pred-keyed) and accumulating STT#2 (gt-keyed)
    def stt1(t):
        sl = sls[t]
        nc.vector.wait_ge(s_pred[t], 16)
        nc.vector.scalar_tensor_tensor(
            r_sb[:, sl], pred_sb[:, sl], C_STAR, pred_sb[:, sl], ge, mx
        ).then_inc(s_r1, 1)

    def stt2(t):
        sl = sls[t]
        nc.vector.wait_ge(s_gt[t], 16)
        nc.vector.wait_ge(s_ln, t + 1)
        return nc.vector.scalar_tensor_tensor(
            r_sb[:, sl], gt_sb[:, sl], 0.0, ln_sb[:, sl], byp, mult,
            accum_out=acc[:, t : t + 1],
        )

    last = None
    for t in range(LEAD):
        stt1(t)
    for t in range(LEAD, NT):
        last = stt2(t - LEAD)
        stt1(t)
    for t in range(NT - LEAD, NT):
        last = stt2(t)
    last.then_inc(s_fin, 1)

    nc.compile()
    return nc


def _get_nc():
    if "nc" not in _CACHE:
        _CACHE["nc"] = _build()
    return _CACHE["nc"]


def _shard(x):
    flat = np.ascontiguousarray(np.asarray(x, dtype=np.float32)).reshape(-1)
    assert flat.size == TOTAL, f"expected {TOTAL} elements, got {flat.size}"
    return [
        flat[c * PER_CORE : (c + 1) * PER_CORE].reshape(P, FREE)
        for c in range(N_CORES)
    ]


def run_spmd(pred, gt, **kw):
    """Shard, run on 8 cores; returns BassKernelResults (kw e.g. trace=True)."""
    preds = _shard(pred)
    gts = _shard(gt)
    in_maps = [{"pred": preds[c], "gt": gts[c]} for c in range(N_CORES)]
    return run_bass_kernel_spmd(
        _get_nc(), in_maps, core_ids=list(range(N_CORES)), **kw
    )


def kernel(pred, gt):
    res = run_spmd(pred, gt)
    total = 0.0
    for r in res.results:
        total += float(r["out"].astype(np.float64).sum())
    return np.asarray(np.float32(-total / TOTAL))
